# revision 1
# baseline (speedup 1.0000x reference)
"""Trainium2 Bass kernel for nn_DNNF (segment_reduce DNF network).

Strategy: data-parallel over batch across 8 NeuronCores (1024 rows each).
The literal axis is host-permuted into 12 phase-planes of 896 columns so the
AND segment-sum (depths cycling [2,4,6]) becomes contiguous vector adds, and
the conjunction axis is ordered group/plane-major so the OR segment-sum is
also contiguous adds. GEMM runs in fp16 on the PE (fp32 PSUM accumulate)
with the tanh applied by the Scalar engine during PSUM eviction.
"""
import numpy as np

import concourse.bacc as bacc
import concourse.mybir as mybir
from concourse import bass_utils
from concourse.tile import TileContext

f32 = mybir.dt.float32
fp16 = mybir.dt.float16
AX = mybir.AxisListType
ALU = mybir.AluOpType
ACTF = mybir.ActivationFunctionType

# problem shape (fixed by the harness)
B, D, L, C, F = 8192, 512, 10752, 2688, 256
NCORES = 8
BS = B // NCORES          # rows per core = 1024
NBT = BS // 128           # b-tiles per core = 8
KT = D // 128             # k-tiles = 4
CW = C // 3               # class width = 896 conj per depth-class
PLANES = L // CW          # 12 literal phase-planes
DEPTHS = (2, 4, 6)
PLANE_BASE = {2: 0, 4: 2, 6: 6}
CLS_OFF = {2: 0, 4: 1, 6: 2}
TEMPERATURE = 2.0

_PROGRAM_CACHE = {}


def _derive_structure(lit2conj, conj2form):
    """Validate the expected DNF structure and return group metadata."""
    depths = np.bincount(lit2conj, minlength=C)
    assert np.array_equal(depths, np.tile(np.array(DEPTHS), C // 3)), \
        "unexpected lit2conj structure"
    cpf = np.bincount(conj2form, minlength=F)
    groups = []          # (formula_start, n_formulas, cpf)
    i = 0
    while i < F:
        j = i
        while j < F and cpf[j] == cpf[i]:
            j += 1
        groups.append((i, j - i, int(cpf[i])))
        i = j
    for (_, nf, c_) in groups:
        assert c_ % 3 == 0, "conj-per-formula not divisible by 3"
    cstart = np.concatenate([[0], np.cumsum(cpf)[:-1]])
    assert np.all(cstart % 3 == 0), "formula conj ranges not 3-aligned"
    return groups, cpf, cstart


def _build_permutation(lit2conj, conj2form, groups, cpf, cstart):
    """Map each literal to its (plane, k) column and conj to class/k index.

    k (0..895) within each depth-class is ordered group-major then
    plane-major then formula-major, which makes both the AND adds
    (literal planes) and the OR adds (conj planes) contiguous.
    """
    conj_depth = np.bincount(lit2conj, minlength=C)
    cls = (np.asarray([CLS_OFF[int(d)] for d in conj_depth]))       # [C]
    # group-class offsets in k-space
    gk0 = {}
    acc = 0
    for gi, (f0, nf, c_) in enumerate(groups):
        gk0[gi] = acc
        acc += nf * (c_ // 3)
    assert acc == CW
    group_of_formula = np.zeros(F, np.int64)
    for gi, (f0, nf, c_) in enumerate(groups):
        group_of_formula[f0:f0 + nf] = gi
    # for each conj: its formula, local formula index, plane j within class
    form_of_conj = np.asarray(conj2form, np.int64)
    g_of_conj = group_of_formula[form_of_conj]
    c3 = np.arange(C) // 3
    s3 = (cstart[form_of_conj] // 3).astype(np.int64)
    j_in_form = c3 - s3                                 # plane within class
    f_local = form_of_conj - np.asarray([groups[g][0] for g in g_of_conj])
    k_of_conj = (np.asarray([gk0[g] for g in g_of_conj])
                 + j_in_form * np.asarray([groups[g][1] for g in g_of_conj])
                 + f_local)
    # literal position within its conj
    first_lit = np.concatenate([[0], np.cumsum(conj_depth)[:-1]])
    lpos = np.arange(L) - first_lit[lit2conj]
    plane = np.asarray([PLANE_BASE[int(d)] for d in conj_depth[lit2conj]]) + lpos
    newcol = plane * CW + k_of_conj[lit2conj]
    assert len(np.unique(newcol)) == L
    inv = np.empty(L, np.int64)
    inv[newcol] = np.arange(L)
    return inv, gk0


def _build_program(groups, gk0, bias_zero):
    key = (tuple(groups), tuple(sorted(gk0.items())), bias_zero)
    if key in _PROGRAM_CACHE:
        return _PROGRAM_CACHE[key]
    assert bias_zero, "nonzero literal bias path not implemented"

    nc = bacc.Bacc("TRN2", target_bir_lowering=False, debug=False,
                   num_devices=NCORES)

    xT_d = nc.dram_tensor("xT", [D, BS], f32, kind="ExternalInput").ap()
    wp_d = nc.dram_tensor("wp", [D, L], f32, kind="ExternalInput").ap()
    mp_d = nc.dram_tensor("mp", [D, L], f32, kind="ExternalInput").ap()
    muT_d = nc.dram_tensor("muT", [D, F], f32, kind="ExternalInput").ap()
    mun_d = nc.dram_tensor("mun", [F, D], f32, kind="ExternalInput").ap()
    sig_d = nc.dram_tensor("sig", [F], f32, kind="ExternalInput").ap()
    eye_d = nc.dram_tensor("eye", [128, 128], fp16, kind="ExternalInput").ap()
    out_d = nc.dram_tensor("out", [BS, F], f32, kind="ExternalOutput").ap()
    scr_d = nc.dram_tensor("m2scr", [F], f32, kind="Internal").ap()

    LN_T = float(np.log(TEMPERATURE))

    with TileContext(nc) as tc:
        with tc.tile_pool(name="cst", bufs=1) as cst, \
             tc.tile_pool(name="stg", bufs=4) as stg, \
             tc.tile_pool(name="stgw", bufs=6) as stgw, \
             tc.tile_pool(name="wrk", bufs=2) as wrk, \
             tc.tile_pool(name="tail", bufs=1) as tail, \
             tc.tile_pool(name="pp", bufs=3) as ppool, \
             tc.tile_pool(name="ps", bufs=2, space="PSUM") as psp:

            # ---------- constants / prep ----------
            bias_cols = {}

            def bias_col(val):
                v = float(val)
                if v not in bias_cols:
                    t = cst.tile([128, 1], f32, tag=f"bc{len(bias_cols)}")
                    nc.vector.memset(t[:], v)
                    bias_cols[v] = t
                return bias_cols[v][:]

            xT_h = cst.tile([128, KT, BS], fp16, tag="xTh")
            muT_h = cst.tile([128, KT, F], fp16, tag="muTh")
            # ---------- pipelined Wm-chunk build + literals ----------
            # chunk-pair outer, b-tile inner: W/M DMA streams under the
            # whole GEMM phase; Wm chunks are transient (never resident).
            conj_all = tail.tile([128, NBT, C], fp16, tag="conj_all")
            form_all = tail.tile([128, NBT, F], f32, tag="form_all")
            acc = {b: tail.tile([128, CW], fp16, tag=f"acc{b}",
                                 name=f"acc{b}")
                   for b in range(NBT)}
            partials = {}
            NC2 = PLANES // 2
            wm_tiles = {}

            def build_chunk(c2):
                t = wrk.tile([128, KT, 2, CW], fp16, tag="wmch")
                for half in range(2):
                    pl = 2 * c2 + half
                    cs = slice(pl * CW, (pl + 1) * CW)
                    for k in range(KT):
                        wst = stgw.tile([128, 1024], f32, tag="stgw")
                        nc.sync.dma_start(wst[:, 0:CW],
                                          wp_d[k * 128:(k + 1) * 128, cs])
                        mst = stgw.tile([128, 1024], f32, tag="stgw")
                        nc.sync.dma_start(mst[:, 0:CW],
                                          mp_d[k * 128:(k + 1) * 128, cs])
                        nc.vector.tensor_mul(t[:, k, half, :], wst[:, 0:CW],
                                             mst[:, 0:CW])
                wm_tiles[c2] = t

            build_chunk(0)
            eye_t = cst.tile([128, 128], fp16, tag="eye")
            nc.gpsimd.dma_start(eye_t[:], eye_d[:])

            # xT: load f32 staging per k-tile, cast to fp16
            for k in range(KT):
                st = stg.tile([128, 1024], f32, tag="stg")
                nc.sync.dma_start(st[:, 0:BS], xT_d[k * 128:(k + 1) * 128, :])
                nc.vector.tensor_copy(xT_h[:, k, :], st[:, 0:BS])
            # muT
            for k in range(KT):
                st = stg.tile([128, 1024], f32, tag="stg")
                nc.gpsimd.dma_start(st[:, 0:F], muT_d[k * 128:(k + 1) * 128, :])
                nc.vector.tensor_copy(muT_h[:, k, :], st[:, 0:F])

            # m2 = sum(mu^2) per formula  ->  DRAM scratch -> row [1, F]
            m2col = cst.tile([128, F // 128], f32, tag="m2col")
            for t in range(F // 128):
                st = stg.tile([128, 1024], f32, tag="stg")
                nc.gpsimd.dma_start(st[:, 0:D], mun_d[t * 128:(t + 1) * 128, :])
                sq = stg.tile([128, 1024], f32, tag="stg")
                nc.vector.tensor_mul(sq[:, 0:D], st[:, 0:D], st[:, 0:D])
                nc.vector.reduce_sum(m2col[:, t:t + 1], sq[:, 0:D],
                                     axis=AX.XYZW)
                nc.gpsimd.dma_start(scr_d[t * 128:(t + 1) * 128],
                                  m2col[:, t:t + 1])
            m2row = cst.tile([1, F], f32, tag="m2row")
            nc.gpsimd.dma_start(m2row[:], scr_d[None, :])

            sigrow = cst.tile([1, F], f32, tag="sigrow")
            nc.gpsimd.dma_start(sigrow[:], sig_d[None, :])
            s2row = cst.tile([1, F], f32, tag="s2row")
            nc.vector.tensor_mul(s2row[:], sigrow[:], sigrow[:])
            arow = cst.tile([1, F], f32, tag="arow")
            nc.vector.reciprocal(arow[:], s2row[:])
            # beta = -0.5*m2 + ln(T)*sigma^2   (so that a*(G+beta-0.5sq)
            #   = a*(G-0.5sq-0.5m2) + ln T)
            t1 = cst.tile([1, F], f32, tag="t1row")
            nc.vector.tensor_scalar_mul(t1[:], m2row[:], -0.5)
            t2 = cst.tile([1, F], f32, tag="t2row")
            nc.vector.tensor_scalar_mul(t2[:], s2row[:], LN_T)
            brow = cst.tile([1, F], f32, tag="brow")
            nc.vector.tensor_add(brow[:], t1[:], t2[:])
            # hi/lo fp16 splits of beta and a
            bhi = cst.tile([1, F], fp16, tag="bhi")
            nc.vector.tensor_copy(bhi[:], brow[:])
            blo32 = cst.tile([1, F], f32, tag="blo32")
            nc.vector.tensor_sub(blo32[:], brow[:], bhi[:])
            blo = cst.tile([1, F], fp16, tag="blo")
            nc.vector.tensor_copy(blo[:], blo32[:])
            ahi = cst.tile([1, F], fp16, tag="ahi")
            nc.vector.tensor_copy(ahi[:], arow[:])
            alo32 = cst.tile([1, F], f32, tag="alo32")
            nc.vector.tensor_sub(alo32[:], arow[:], ahi[:])
            alo = cst.tile([1, F], fp16, tag="alo")
            nc.vector.tensor_copy(alo[:], alo32[:])
            # beta2 / a2: [2, F] fp16 via tiny DMA through scratch is
            # avoided — use two K=1 matmuls instead (ones row x row).
            ones1 = cst.tile([1, 128], fp16, tag="ones1")
            nc.vector.memset(ones1[:], 1.0)


            for c2 in range(NC2):
                if c2 + 1 < NC2:
                    build_chunk(c2 + 1)
                wmc = wm_tiles.pop(c2)
                for b in range(NBT):
                    bs = slice(b * 128, (b + 1) * 128)
                    ps_l = psp.tile([128, 2048], f32, tag="ps")
                    for half in range(2):
                        for (o0, w_) in ((0, 512), (512, 384)):
                            po = half * 1024 + o0
                            for k in range(KT):
                                nc.tensor.matmul(
                                    ps_l[:, po:po + w_], xT_h[:, k, bs],
                                    wmc[:, k, half, o0:o0 + w_],
                                    start=(k == 0), stop=(k == KT - 1))
                    ev = ppool.tile([128, 2, CW], fp16, tag="ev")
                    pv = ps_l[:].rearrange("p (h w) -> p h w", h=2)
                    nc.scalar.activation(ev[:], pv[:, :, 0:CW], ACTF.Tanh)
                    # incremental AND-stage adds per depth-class
                    if c2 == 0:
                        nc.vector.tensor_add(conj_all[:, b, 0:CW],
                                             ev[:, 0, :], ev[:, 1, :])
                    elif c2 == 1:
                        nc.vector.tensor_add(acc[b][:], ev[:, 0, :],
                                             ev[:, 1, :])
                    elif c2 == 2:
                        t2 = ppool.tile([128, CW], fp16, tag="evs")
                        nc.vector.tensor_add(t2[:], ev[:, 0, :], ev[:, 1, :])
                        nc.vector.tensor_add(conj_all[:, b, CW:2 * CW],
                                             acc[b][:], t2[:])
                    elif c2 == 3:
                        nc.vector.tensor_add(acc[b][:], ev[:, 0, :],
                                             ev[:, 1, :])
                    elif c2 == 4:
                        t2 = ppool.tile([128, CW], fp16, tag="evs")
                        nc.vector.tensor_add(t2[:], ev[:, 0, :], ev[:, 1, :])
                        nc.vector.tensor_add(acc[b][:], acc[b][:], t2[:])
                    else:
                        t2 = ppool.tile([128, CW], fp16, tag="evs")
                        nc.vector.tensor_add(t2[:], ev[:, 0, :], ev[:, 1, :])
                        nc.vector.tensor_add(conj_all[:, b, 2 * CW:3 * CW],
                                             acc[b][:], t2[:])
                    # conj tanh once a class is complete for this b
                    for ci, (c2done, d) in enumerate(zip((0, 2, 5), DEPTHS)):
                        if c2 == c2done:
                            sl = conj_all[:, b, ci * CW:(ci + 1) * CW]
                            nc.scalar.activation(sl, sl, ACTF.Tanh,
                                                 bias=bias_col(1.5 - d))
                # form partials for the class that just completed (all b)
                ci = {0: 0, 2: 1, 5: 2}.get(c2)
                if ci is not None:
                    for gi, (f0, nf, cpf_g) in enumerate(groups):
                        m = cpf_g // 3
                        k0 = ci * CW + gk0[gi]
                        sl = [conj_all[:, :, k0 + j * nf:k0 + (j + 1) * nf]
                              for j in range(m)]
                        p = tail.tile([128, NBT, 64], fp16,
                                      tag=f"pt{gi}_{ci}")
                        pv_ = p[:, :, 0:nf]
                        if m == 2:
                            nc.vector.tensor_add(pv_, sl[0], sl[1])
                        elif m == 3:
                            nc.vector.tensor_add(pv_, sl[0], sl[1])
                            nc.vector.tensor_add(pv_, pv_, sl[2])
                        elif m == 4:
                            tmp = ppool.tile([128, NBT, 64], fp16, tag="fpt")
                            tmpv = tmp[:, :, 0:nf]
                            nc.vector.tensor_add(pv_, sl[0], sl[1])
                            nc.vector.tensor_add(tmpv, sl[2], sl[3])
                            nc.vector.tensor_add(pv_, pv_, tmpv)
                        elif m == 5:
                            tmp = ppool.tile([128, NBT, 64], fp16, tag="fpt")
                            tmpv = tmp[:, :, 0:nf]
                            nc.vector.tensor_add(pv_, sl[0], sl[1])
                            nc.vector.tensor_add(tmpv, sl[2], sl[3])
                            nc.vector.tensor_add(pv_, pv_, tmpv)
                            nc.vector.tensor_add(pv_, pv_, sl[4])
                        else:
                            raise AssertionError(f"unsupported cpf {cpf_g}")
                        partials[(gi, ci)] = pv_

                if c2 == 2:
                    # A_bc = broadcast of a (1/sigma^2) to [128, F] via 2 rank-1 mms
                    ps_bc = psp.tile([128, 2048], f32, tag="ps")
                    nc.tensor.matmul(ps_bc[:, 0:F], ones1[:], ahi[:],
                                     start=True, stop=False)
                    nc.tensor.matmul(ps_bc[:, 0:F], ones1[:], alo[:],
                                     start=False, stop=True)
                    a_bc = cst.tile([128, F], f32, tag="a_bc")
                    nc.vector.tensor_copy(a_bc[:], ps_bc[:, 0:F])

                    # ---------- loc branch: G2 psums + Gram diag ----------
                    sq_all = cst.tile([128, NBT], f32, tag="sq_all")
                    sqh_all = cst.tile([128, NBT], f32, tag="sqh_all")
                    z_all = tail.tile([128, NBT, F], fp16, tag="z_all")
                    for b in range(NBT):
                        bs = slice(b * 128, (b + 1) * 128)
                        # Gram diag for ||x||^2
                        ps_g = psp.tile([128, 2048], f32, tag="ps")
                        for k in range(KT):
                            nc.tensor.matmul(ps_g[:, 0:128], xT_h[:, k, bs],
                                             xT_h[:, k, bs],
                                             start=(k == 0), stop=(k == KT - 1))
                        gd = stg.tile([128, 1024], f32, tag="stg")
                        nc.vector.tensor_mul(gd[:, 0:128], ps_g[:, 0:128], eye_t[:])
                        nc.vector.reduce_sum(sq_all[:, b:b + 1], gd[:, 0:128],
                                             axis=AX.XYZW)
                        nc.vector.tensor_scalar_mul(sqh_all[:, b:b + 1],
                                                    sq_all[:, b:b + 1], 0.5)
                        # G2 + rank-1 beta
                        ps_G = psp.tile([128, 2048], f32, tag="ps")
                        for k in range(KT):
                            nc.tensor.matmul(ps_G[:, 0:F], xT_h[:, k, bs],
                                             muT_h[:, k, :],
                                             start=(k == 0), stop=False)
                        nc.tensor.matmul(ps_G[:, 0:F], ones1[:], bhi[:],
                                         start=False, stop=False)
                        nc.tensor.matmul(ps_G[:, 0:F], ones1[:], blo[:],
                                         start=False, stop=True)
                        # z = a * (G + beta - 0.5*||x||^2)   [-> T*exp(w) after Exp]
                        nc.vector.scalar_tensor_tensor(
                            z_all[:, b, :], ps_G[:, 0:F], sqh_all[:, b:b + 1],
                            a_bc[:], op0=ALU.subtract, op1=ALU.mult)

                    e_t = tail.tile([128, NBT, F], fp16, tag="e_t")
                    nc.scalar.activation(z_all[:], z_all[:], ACTF.Exp)
                    nc.scalar.activation(e_t[:], z_all[:], ACTF.Exp)
                    s_t = tail.tile([128, NBT], f32, tag="s_t")
                    nc.vector.reduce_sum(s_t[:], e_t[:], axis=AX.X)
                    r_t = tail.tile([128, NBT], f32, tag="r_t")
                    nc.vector.reciprocal(r_t[:], s_t[:])


            # ---------- combine partials + dnnf ----------
            for gi, (f0, nf, cpf_g) in enumerate(groups):
                fv = form_all[:, :, f0:f0 + nf]
                tmpf = ppool.tile([128, NBT, 64], f32, tag="fpf32")
                tmpfv = tmpf[:, :, 0:nf]
                nc.vector.tensor_add(tmpfv, partials[(gi, 0)],
                                     partials[(gi, 1)])
                nc.vector.tensor_add(fv, tmpfv, partials[(gi, 2)])
                nc.scalar.activation(fv, fv, ACTF.Tanh,
                                     bias=bias_col(cpf_g - 1.5))

            # ---------- softmax tail (exp/sum/recip ran mid-loop) ----------
            nc.vector.tensor_mul(form_all[:], form_all[:], e_t[:])
            for b in range(NBT):
                nc.vector.tensor_scalar_mul(form_all[:, b, :],
                                            form_all[:, b, :],
                                            r_t[:, b:b + 1])
            nc.sync.dma_start(out_d.rearrange("(b p) f -> p b f", p=128),
                              form_all[:])

    nc.compile()
    _PROGRAM_CACHE[key] = nc
    return nc


def kernel(x, weight, learnable_binary_mask, bias, mu, sigma,
           lit2conj, conj2form):
    x = np.asarray(x, np.float32)
    weight = np.asarray(weight, np.float32)
    mask = np.asarray(learnable_binary_mask, np.float32)
    bias = np.asarray(bias, np.float32)
    mu = np.asarray(mu, np.float32)
    sigma = np.asarray(sigma, np.float32)
    lit2conj = np.asarray(lit2conj, np.int64)
    conj2form = np.asarray(conj2form, np.int64)

    groups, cpf, cstart = _derive_structure(lit2conj, conj2form)
    inv, gk0 = _build_permutation(lit2conj, conj2form, groups, cpf, cstart)
    bias_zero = bool(np.all(bias == 0))

    nc = _build_program(groups, gk0, bias_zero)

    wp = np.ascontiguousarray(weight[:, inv])
    mp = np.ascontiguousarray(mask[:, inv])
    muT = np.ascontiguousarray(mu.T)
    eye = np.eye(128, dtype=np.float16)

    in_maps = []
    for i in range(NCORES):
        xs = x[i * BS:(i + 1) * BS]
        in_maps.append({
            "xT": np.ascontiguousarray(xs.T),
            "wp": wp, "mp": mp, "muT": muT, "mun": mu,
            "sig": sigma, "eye": eye,
        })

    res = bass_utils.run_bass_kernel_spmd(nc, in_maps,
                                          core_ids=list(range(NCORES)))
    out = np.concatenate([res.results[i]["out"] for i in range(NCORES)],
                         axis=0)
    return out.astype(np.float32)



# revision 6
# speedup vs baseline: 1.0572x; 1.0572x over previous
"""Trainium2 Bass kernel for nn_DNNF (segment_reduce DNF network).

Strategy: data-parallel over batch across 8 NeuronCores (1024 rows each).
The literal axis is host-permuted into 12 phase-planes of 896 columns so the
AND segment-sum (depths cycling [2,4,6]) becomes contiguous vector adds, and
the conjunction axis is ordered group/plane-major so the OR segment-sum is
also contiguous adds. GEMM runs in fp16 on the PE (fp32 PSUM accumulate)
with the tanh applied by the Scalar engine during PSUM eviction.

v2: localization branch runs first (overlaps W/M streaming + PE warmup),
W/M arrive as fp16 via casting SWDGE DMAs and are multiplied in place on
Vector, matmuls are k-outer (stationary reuse), beta/a/or_bias are SBUF
broadcasts built once via rank-1 matmuls, and the last chunk's epilogue
(conj tanh, OR reduce, softmax multiply, output DMA) is interleaved per
b-tile so the kernel tail is one b-tile deep.
"""
import numpy as np

import concourse.bacc as bacc
import concourse.mybir as mybir
from concourse import bass_utils
from concourse.tile import TileContext

f32 = mybir.dt.float32
fp16 = mybir.dt.float16
AX = mybir.AxisListType
ALU = mybir.AluOpType
ACTF = mybir.ActivationFunctionType

# problem shape (fixed by the harness)
B, D, L, C, F = 8192, 512, 10752, 2688, 256
NCORES = 8
BS = B // NCORES          # rows per core = 1024
NBT = BS // 128           # b-tiles per core = 8
KT = D // 128             # k-tiles = 4
CW = C // 3               # class width = 896 conj per depth-class
PLANES = L // CW          # 12 literal phase-planes
DEPTHS = (2, 4, 6)
PLANE_BASE = {2: 0, 4: 2, 6: 6}
CLS_OFF = {2: 0, 4: 1, 6: 2}
TEMPERATURE = 2.0
NC2 = PLANES // 2         # 6 chunk-pairs
# chunk-pair -> (class index completed by this chunk, running-sum mode)
# mode: 'w' write sum of the two evicted planes, 'a' accumulate onto slot
CHUNK_CLS = [(0, 'w'), (1, 'w'), (1, 'a'), (2, 'w'), (2, 'a'), (2, 'a')]
CLS_DONE = {0: 0, 2: 1, 5: 2}   # chunk -> class completed

_PROGRAM_CACHE = {}


def _derive_structure(lit2conj, conj2form):
    """Validate the expected DNF structure and return group metadata."""
    depths = np.bincount(lit2conj, minlength=C)
    assert np.array_equal(depths, np.tile(np.array(DEPTHS), C // 3)), \
        "unexpected lit2conj structure"
    cpf = np.bincount(conj2form, minlength=F)
    groups = []          # (formula_start, n_formulas, cpf)
    i = 0
    while i < F:
        j = i
        while j < F and cpf[j] == cpf[i]:
            j += 1
        groups.append((i, j - i, int(cpf[i])))
        i = j
    for (_, nf, c_) in groups:
        assert c_ % 3 == 0, "conj-per-formula not divisible by 3"
    cstart = np.concatenate([[0], np.cumsum(cpf)[:-1]])
    assert np.all(cstart % 3 == 0), "formula conj ranges not 3-aligned"
    return groups, cpf, cstart


def _build_permutation(lit2conj, conj2form, groups, cpf, cstart):
    """Map each literal to its (plane, k) column and conj to class/k index.

    k (0..895) within each depth-class is ordered group-major then
    plane-major then formula-major, which makes both the AND adds
    (literal planes) and the OR adds (conj planes) contiguous.
    """
    conj_depth = np.bincount(lit2conj, minlength=C)
    # group-class offsets in k-space
    gk0 = {}
    acc = 0
    for gi, (f0, nf, c_) in enumerate(groups):
        gk0[gi] = acc
        acc += nf * (c_ // 3)
    assert acc == CW
    group_of_formula = np.zeros(F, np.int64)
    for gi, (f0, nf, c_) in enumerate(groups):
        group_of_formula[f0:f0 + nf] = gi
    form_of_conj = np.asarray(conj2form, np.int64)
    g_of_conj = group_of_formula[form_of_conj]
    c3 = np.arange(C) // 3
    s3 = (cstart[form_of_conj] // 3).astype(np.int64)
    j_in_form = c3 - s3                                 # plane within class
    f_local = form_of_conj - np.asarray([groups[g][0] for g in g_of_conj])
    k_of_conj = (np.asarray([gk0[g] for g in g_of_conj])
                 + j_in_form * np.asarray([groups[g][1] for g in g_of_conj])
                 + f_local)
    first_lit = np.concatenate([[0], np.cumsum(conj_depth)[:-1]])
    lpos = np.arange(L) - first_lit[lit2conj]
    plane = np.asarray([PLANE_BASE[int(d)] for d in conj_depth[lit2conj]]) + lpos
    newcol = plane * CW + k_of_conj[lit2conj]
    assert len(np.unique(newcol)) == L
    inv = np.empty(L, np.int64)
    inv[newcol] = np.arange(L)
    return inv, gk0


def _build_program(groups, gk0, bias_zero):
    key = (tuple(groups), tuple(sorted(gk0.items())), bias_zero)
    if key in _PROGRAM_CACHE:
        return _PROGRAM_CACHE[key]
    assert bias_zero, "nonzero literal bias path not implemented"

    nc = bacc.Bacc("TRN2", target_bir_lowering=False, debug=False,
                   num_devices=NCORES)

    xT_d = nc.dram_tensor("xT", [D, BS], f32, kind="ExternalInput").ap()
    wp_d = nc.dram_tensor("wp", [D, L], f32, kind="ExternalInput").ap()
    mp_d = nc.dram_tensor("mp", [D, L], f32, kind="ExternalInput").ap()
    muT_d = nc.dram_tensor("muT", [D, F], f32, kind="ExternalInput").ap()
    mun_d = nc.dram_tensor("mun", [F, D], f32, kind="ExternalInput").ap()
    sig_d = nc.dram_tensor("sig", [F], f32, kind="ExternalInput").ap()
    eyeh_d = nc.dram_tensor("eyeh", [128, 128], fp16, kind="ExternalInput").ap()
    out_d = nc.dram_tensor("out", [BS, F], f32, kind="ExternalOutput").ap()
    scr_d = nc.dram_tensor("m2scr", [F], f32, kind="Internal").ap()

    LN_T = float(np.log(TEMPERATURE))
    out_v = out_d.rearrange("(b p) f -> p b f", p=128)

    with TileContext(nc) as tc:
        with tc.tile_pool(name="cst", bufs=1) as cst, \
             tc.tile_pool(name="stg", bufs=2) as stg, \
             tc.tile_pool(name="wchk", bufs=3) as wchk, \
             tc.tile_pool(name="mpc", bufs=6) as mpc, \
             tc.tile_pool(name="ev", bufs=3) as evp, \
             tc.tile_pool(name="fin", bufs=2) as finp, \
             tc.tile_pool(name="ps", bufs=2, space="PSUM") as psp:

            bias_cols = {}

            def bias_col(val):
                v = float(val)
                if v not in bias_cols:
                    t = cst.tile([128, 1], f32, tag=f"bc{len(bias_cols)}",
                                 name=f"bc{len(bias_cols)}")
                    nc.vector.memset(t[:], v)
                    bias_cols[v] = t
                return bias_cols[v][:]

            # ---------- input DMAs (issue order = ring order) ----------
            # SWDGE (gpsimd) ring: casting f32->fp16 loads.
            xT_h = cst.tile([128, KT, BS], fp16, tag="xTh")
            nc.gpsimd.dma_start(
                xT_h[:], xT_d.rearrange("(k p) b -> p k b", p=128))
            muT_h = cst.tile([128, KT, F], fp16, tag="muTh")
            nc.gpsimd.dma_start(
                muT_h[:], muT_d.rearrange("(k p) f -> p k f", p=128))
            # HWDGE (sync) ring: small f32 loads for the beta/a rows.
            mun_t = {}
            for t in range(F // 128):
                mt = stg.tile([128, D], f32, tag="mun")
                nc.sync.dma_start(mt[:], mun_d[t * 128:(t + 1) * 128, :])
                mun_t[t] = mt
            sigrow = cst.tile([1, F], f32, tag="sigrow")
            nc.sync.dma_start(sigrow[:], sig_d[None, :])
            eyeh_t = cst.tile([128, 128], fp16, tag="eyeh")
            nc.sync.dma_start(eyeh_t[:], eyeh_d[:])

            # W/M chunk pieces: casting SWDGE DMAs, chunk 0 first.
            wm_tiles = {}
            mpieces = {}

            def issue_chunk_dma(c2):
                wt = wchk.tile([128, KT, 2 * CW], fp16, tag="wch")
                wm_tiles[c2] = wt
                cs = slice(2 * c2 * CW, (2 * c2 + 2) * CW)
                for k in range(KT):
                    nc.gpsimd.dma_start(wt[:, k, :],
                                        wp_d[k * 128:(k + 1) * 128, cs])
                    mt = mpc.tile([128, 2 * CW], fp16, tag="mpc")
                    nc.gpsimd.dma_start(mt[:],
                                        mp_d[k * 128:(k + 1) * 128, cs])
                    mpieces[(c2, k)] = mt

            issue_chunk_dma(0)
            issue_chunk_dma(1)

            # ---------- beta/a rows + broadcasts ----------
            # m2 = sum(mu^2) per formula -> DRAM scratch -> row [1, F]
            m2col = cst.tile([128, F // 128], f32, tag="m2col")
            for t in range(F // 128):
                sq = stg.tile([128, D], f32, tag="musq")
                nc.vector.tensor_mul(sq[:], mun_t[t][:], mun_t[t][:])
                nc.vector.reduce_sum(m2col[:, t:t + 1], sq[:], axis=AX.XYZW)
                nc.sync.dma_start(scr_d[t * 128:(t + 1) * 128],
                                  m2col[:, t:t + 1])
            m2row = cst.tile([1, F], f32, tag="m2row")
            nc.sync.dma_start(m2row[:], scr_d[None, :])

            s2row = cst.tile([1, F], f32, tag="s2row")
            nc.vector.tensor_mul(s2row[:], sigrow[:], sigrow[:])
            arow = cst.tile([1, F], f32, tag="arow")
            nc.vector.reciprocal(arow[:], s2row[:])
            # beta = -0.5*m2 + ln(T)*sigma^2 ; ab = a*beta
            t1 = cst.tile([1, F], f32, tag="t1row")
            nc.vector.tensor_scalar_mul(t1[:], m2row[:], -0.5)
            t2 = cst.tile([1, F], f32, tag="t2row")
            nc.vector.tensor_scalar_mul(t2[:], s2row[:], LN_T)
            brow = cst.tile([1, F], f32, tag="brow")
            nc.vector.tensor_add(brow[:], t1[:], t2[:])
            abrow = cst.tile([1, F], f32, tag="abrow")
            nc.vector.tensor_mul(abrow[:], arow[:], brow[:])

            # hi/lo fp16 splits for exact rank-1 broadcast
            def hilo(row, tagp):
                hi = cst.tile([1, F], fp16, tag=tagp + "hi")
                nc.vector.tensor_copy(hi[:], row[:])
                lo32 = cst.tile([1, F], f32, tag=tagp + "lo32")
                nc.vector.tensor_sub(lo32[:], row[:], hi[:])
                lo = cst.tile([1, F], fp16, tag=tagp + "lo")
                nc.vector.tensor_copy(lo[:], lo32[:])
                return hi, lo

            ahi, alo = hilo(arow, "a")
            abhi, ablo = hilo(abrow, "ab")
            ones1 = cst.tile([1, 128], fp16, tag="ones1")
            nc.vector.memset(ones1[:], 1.0)
            # or_bias row (cpf - 1.5 per formula) is fp16-exact
            obrow = cst.tile([1, F], fp16, tag="obrow")
            for gi, (f0, nf, cpf_g) in enumerate(groups):
                nc.vector.memset(obrow[:, f0:f0 + nf], float(cpf_g) - 1.5)

            ps_bc = psp.tile([128, 2048], f32, tag="ps")
            nc.tensor.matmul(ps_bc[:, 0:F], ones1[:], ahi[:],
                             start=True, stop=False)
            nc.tensor.matmul(ps_bc[:, 0:F], ones1[:], alo[:],
                             start=False, stop=True)
            nc.tensor.matmul(ps_bc[:, 1024:1024 + F], ones1[:], abhi[:],
                             start=True, stop=False)
            nc.tensor.matmul(ps_bc[:, 1024:1024 + F], ones1[:], ablo[:],
                             start=False, stop=True)
            a_bc = cst.tile([128, F], f32, tag="a_bc")
            nc.vector.tensor_copy(a_bc[:], ps_bc[:, 0:F])
            ab_bc = cst.tile([128, F], f32, tag="ab_bc")
            nc.vector.tensor_copy(ab_bc[:], ps_bc[:, 1024:1024 + F])
            ps_bc2 = psp.tile([128, 2048], f32, tag="ps")
            nc.tensor.matmul(ps_bc2[:, 0:F], ones1[:], obrow[:],
                             start=True, stop=True)
            ob_bc = cst.tile([128, F], f32, tag="ob_bc")
            nc.vector.tensor_copy(ob_bc[:], ps_bc2[:, 0:F])

            # ---------- localization branch (first PE phase) ----------
            z_all = cst.tile([128, NBT, F], fp16, tag="z_all")
            e_t = cst.tile([128, NBT, F], fp16, tag="e_t")
            sqh_all = cst.tile([128, NBT], f32, tag="sqh_all")
            for b in range(NBT):
                bs = slice(b * 128, (b + 1) * 128)
                ps_G = psp.tile([128, 2048], f32, tag="ps")
                for k in range(KT):
                    nc.tensor.matmul(ps_G[:, 0:F], xT_h[:, k, bs],
                                     muT_h[:, k, :],
                                     start=(k == 0), stop=(k == KT - 1))
                    nc.tensor.matmul(ps_G[:, 512:640], xT_h[:, k, bs],
                                     xT_h[:, k, bs],
                                     start=(k == 0), stop=(k == KT - 1))
                # 0.5*||x||^2 via half-eye Gram diag
                gd = stg.tile([128, 128], f32, tag="gd")
                nc.vector.tensor_mul(gd[:], ps_G[:, 512:640], eyeh_t[:])
                nc.vector.reduce_sum(sqh_all[:, b:b + 1], gd[:], axis=AX.XYZW)
                # z = a*(G - 0.5||x||^2) + a*beta
                zf = stg.tile([128, F], f32, tag="zf")
                nc.vector.scalar_tensor_tensor(
                    zf[:], ps_G[:, 0:F], sqh_all[:, b:b + 1], a_bc[:],
                    op0=ALU.subtract, op1=ALU.mult)
                nc.vector.tensor_add(z_all[:, b, :], zf[:], ab_bc[:])
            nc.scalar.activation(z_all[:], z_all[:], ACTF.Exp)
            nc.scalar.activation(e_t[:], z_all[:], ACTF.Exp)
            s_t = cst.tile([128, NBT], f32, tag="s_t")
            nc.vector.reduce_sum(s_t[:], e_t[:], axis=AX.X)
            r_t = cst.tile([128, NBT], f32, tag="r_t")
            nc.vector.reciprocal(r_t[:], s_t[:])

            # ---------- literal GEMM over 6 chunk-pairs ----------
            conj_all = cst.tile([128, NBT, C], fp16, tag="conj_all")
            partials = {ci: cst.tile([128, NBT, F], fp16, tag=f"part{ci}",
                                     name=f"part{ci}")
                        for ci in range(3)}

            def or_reduce(ci, b_sl, p_sl):
                """OR-stage partial sums for class ci, batched over b_sl."""
                for gi, (f0, nf, cpf_g) in enumerate(groups):
                    m = cpf_g // 3
                    k0 = ci * CW + gk0[gi]
                    sl = [conj_all[:, b_sl, k0 + j * nf:k0 + (j + 1) * nf]
                          for j in range(m)]
                    pv = partials[ci][:, p_sl, f0:f0 + nf]
                    nc.vector.tensor_add(pv, sl[0], sl[1])
                    for j in range(2, m):
                        nc.vector.tensor_add(pv, pv, sl[j])

            def finish_b(b):
                """Combine partials -> form tanh -> softmax multiply -> DMA."""
                t2f = finp.tile([128, F], f32, tag="t2f")
                nc.vector.tensor_add(t2f[:], partials[0][:, b, :],
                                     partials[1][:, b, :])
                nc.vector.tensor_add(t2f[:], t2f[:], partials[2][:, b, :])
                fpre = finp.tile([128, F], f32, tag="fpre")
                nc.vector.tensor_add(fpre[:], t2f[:], ob_bc[:])
                nc.scalar.activation(fpre[:], fpre[:], ACTF.Tanh)
                ot = finp.tile([128, F], f32, tag="ot")
                nc.vector.scalar_tensor_tensor(
                    ot[:], fpre[:], r_t[:, b:b + 1], e_t[:, b, :],
                    op0=ALU.mult, op1=ALU.mult)
                nc.sync.dma_start(out_v[:, b, :], ot[:])

            for c2 in range(NC2):
                if c2 + 2 < NC2:
                    issue_chunk_dma(c2 + 2)
                wt = wm_tiles.pop(c2)
                for k in range(KT):
                    mt = mpieces.pop((c2, k))
                    nc.vector.tensor_mul(wt[:, k, :], wt[:, k, :], mt[:])
                ci, mode = CHUNK_CLS[c2]
                cls_sl = slice(ci * CW, (ci + 1) * CW)
                last = (c2 == NC2 - 1)
                for b in range(NBT):
                    bs = slice(b * 128, (b + 1) * 128)
                    ps_l = psp.tile([128, 2048], f32, tag="ps")
                    for k in range(KT):
                        for half in range(2):
                            for (o0, w_) in ((0, 512), (512, 384)):
                                po = half * 1024 + o0
                                nc.tensor.matmul(
                                    ps_l[:, po:po + w_], xT_h[:, k, bs],
                                    wt[:, k, half * CW + o0:half * CW + o0 + w_],
                                    start=(k == 0), stop=(k == KT - 1))
                    ev = evp.tile([128, 2, CW], fp16, tag="ev")
                    pv = ps_l[:].rearrange("p (h w) -> p h w", h=2)
                    nc.scalar.activation(ev[:], pv[:, :, 0:CW], ACTF.Tanh)
                    if mode == 'w':
                        nc.vector.tensor_add(conj_all[:, b, cls_sl],
                                             ev[:, 0, :], ev[:, 1, :])
                    else:
                        t_ = evp.tile([128, CW], fp16, tag="evs")
                        nc.vector.tensor_add(t_[:], ev[:, 0, :], ev[:, 1, :])
                        nc.vector.tensor_add(conj_all[:, b, cls_sl],
                                             conj_all[:, b, cls_sl], t_[:])
                    if last:
                        # per-b epilogue: conj tanh class 2, OR reduce,
                        # combine + softmax multiply + output DMA
                        sl = conj_all[:, b, cls_sl]
                        nc.scalar.activation(sl, sl, ACTF.Tanh,
                                             bias=bias_col(1.5 - DEPTHS[ci]))
                        or_reduce(ci, b, b)
                        finish_b(b)
                cdone = CLS_DONE.get(c2)
                if cdone is not None and not last:
                    dsl = slice(cdone * CW, (cdone + 1) * CW)
                    sl = conj_all[:, :, dsl]
                    nc.scalar.activation(sl, sl, ACTF.Tanh,
                                         bias=bias_col(1.5 - DEPTHS[cdone]))
                    or_reduce(cdone, slice(0, NBT), slice(0, NBT))

    nc.compile()
    _PROGRAM_CACHE[key] = nc
    return nc


def kernel(x, weight, learnable_binary_mask, bias, mu, sigma,
           lit2conj, conj2form):
    x = np.asarray(x, np.float32)
    weight = np.asarray(weight, np.float32)
    mask = np.asarray(learnable_binary_mask, np.float32)
    bias = np.asarray(bias, np.float32)
    mu = np.asarray(mu, np.float32)
    sigma = np.asarray(sigma, np.float32)
    lit2conj = np.asarray(lit2conj, np.int64)
    conj2form = np.asarray(conj2form, np.int64)

    groups, cpf, cstart = _derive_structure(lit2conj, conj2form)
    inv, gk0 = _build_permutation(lit2conj, conj2form, groups, cpf, cstart)
    bias_zero = bool(np.all(bias == 0))

    nc = _build_program(groups, gk0, bias_zero)

    wp = np.ascontiguousarray(weight[:, inv])
    mp = np.ascontiguousarray(mask[:, inv])
    muT = np.ascontiguousarray(mu.T)
    eyeh = (0.5 * np.eye(128)).astype(np.float16)

    in_maps = []
    for i in range(NCORES):
        xs = x[i * BS:(i + 1) * BS]
        in_maps.append({
            "xT": np.ascontiguousarray(xs.T),
            "wp": wp, "mp": mp, "muT": muT, "mun": mu,
            "sig": sigma, "eyeh": eyeh,
        })

    res = bass_utils.run_bass_kernel_spmd(nc, in_maps,
                                          core_ids=list(range(NCORES)))
    out = np.concatenate([res.results[i]["out"] for i in range(NCORES)],
                         axis=0)
    return out.astype(np.float32)
